# revision 3
# baseline (speedup 1.0000x reference)
"""Trainium2 Bass kernel for MultiHeadQuadraticStateSpaceAttention.

Reference computation (B=4, T=4096, E=1024, H=16, S=64, D=64):
    a_state[h,b,s]   = state[h,b,:] @ A_w[h].T + A_b[h]                  (tiny)
    proj[b,t,h,s]    = sum_d sm[h,s,d] * x[b,t,h*64+d]
    new_state        = proj + a_state (broadcast over t)                 -> output [H,B,T,S]
    out_h[b,t,h,d]   = sum_s new_state[b,t,h,s] * C_w[h,d,s] + C_b[h,d]
    out              = out_h.reshape(B,T,E) @ Wo.T + bo                  -> output [B,T,E]

Kernel strategy (8 NeuronCores, data-parallel over the 16384 tokens):
  * core c handles batch b = c//2 and the token half t in [(c%2)*2048, ...+2048)
  * C_w and Wo are algebraically fused on the host into one [E_in=(h,s), E_out]
    weight: out = new_state.reshape(t, H*S) @ W2 + bo_eff, with
    W2[(h,s), e] = sum_d C_w[h,d,s] * Wo[e, h*64+d],
    bo_eff = bo + Wo @ C_b.flatten()  (exact algebraic identity).
  * per 512-token stripe on each core:
      - DMA x stripe [512, 1024]
      - PE-transpose x into feature-major xT [(h,d), t] (fp32, exact),
        PSUM->SBUF copy rounds to float32r
      - mm1 (f32r): per 2-head group g, ns_T[(h,s), t] = smT_bd[g].T @ xT[g]
        (block-diagonal sm^T packs 2 heads into one K=128 matmul)
      - DVE copies psum->SBUF twice, adding a_state per-partition:
        nsT (fp32, for the exact new_state output) and nsT_r (f32r, for mm3)
      - PE-transpose nsT back to token-major, store new_state to HBM
      - mm3 (f32r): out[t, e] = sum_g nsT_r[g].T @ W2[g], accumulated in PSUM
      - ScalarE copies out psum->SBUF, store to HBM
"""

import sys

if "/opt/trn_rl_repo" not in sys.path:
    sys.path.insert(0, "/opt/trn_rl_repo")

import numpy as np

import concourse.bacc as bacc
import concourse.tile as tile
from concourse import mybir
from concourse.bass import ds, ts
from concourse.bass_utils import run_bass_kernel_spmd

F32 = mybir.dt.float32
F32R = mybir.dt.float32r

B, T, E, H, S = 4, 4096, 1024, 16, 64
D = E // H
NCORES = 8
T_LOC = (B * T) // NCORES          # 2048 tokens per core
P = 128
NG = H // 2                        # 8 two-head groups (2*64 = 128 = K)
STRIPE = 512
NSTRIPES = T_LOC // STRIPE         # 4
TT_PER_STRIPE = STRIPE // P        # 4

_CACHE = {}
_last_in_maps = None


def _build_kernel(with_bias: bool):
    nc = bacc.Bacc(None, target_bir_lowering=False, debug=False)

    x_in = nc.dram_tensor("x_in", [T_LOC, E], F32, kind="ExternalInput")
    ident_in = nc.dram_tensor("ident_in", [P, P], F32, kind="ExternalInput")
    smT_in = nc.dram_tensor("smT_in", [NG, P, P], F32R, kind="ExternalInput")
    w2_in = nc.dram_tensor("w2_in", [NG, P, E], F32R, kind="ExternalInput")
    awT_in = nc.dram_tensor("awT_in", [NG, P, P], F32, kind="ExternalInput")
    state_in = nc.dram_tensor("state_in", [P, NG], F32, kind="ExternalInput")
    ab_in = nc.dram_tensor("ab_in", [P, NG], F32, kind="ExternalInput")
    if with_bias:
        ones_in = nc.dram_tensor("ones_in", [1, P], F32R, kind="ExternalInput")
        bo_in = nc.dram_tensor("bo_in", [1, E], F32R, kind="ExternalInput")

    out_o = nc.dram_tensor("out_o", [T_LOC, E], F32, kind="ExternalOutput")
    ns_o = nc.dram_tensor("ns_o", [H, T_LOC, S], F32, kind="ExternalOutput")

    x_view = x_in.rearrange("(st o p) e -> st p o e", p=P, o=TT_PER_STRIPE)

    with tile.TileContext(nc) as tc:
        with (
            tc.tile_pool(name="consts", bufs=1) as consts,
            tc.tile_pool(name="xs", bufs=2) as xs,
            tc.tile_pool(name="xts", bufs=2) as xts,
            tc.tile_pool(name="nsts", bufs=2) as nsts,
            tc.tile_pool(name="stores", bufs=3) as stores,
            tc.tile_pool(name="ptx", bufs=2, space="PSUM") as ptx_pool,
            tc.tile_pool(name="pmm1", bufs=2, space="PSUM") as pmm1_pool,
            tc.tile_pool(name="ptns", bufs=2, space="PSUM") as ptns_pool,
            tc.tile_pool(name="pmm3", bufs=2, space="PSUM") as pmm3_pool,
        ):
            ident = consts.tile([P, P], F32)
            nc.sync.dma_start(ident[:], ident_in[:])
            smT = consts.tile([P, NG, P], F32R)
            nc.sync.dma_start(smT[:], smT_in.rearrange("g p c -> p g c"))
            w2 = consts.tile([P, NG, E], F32R)
            nc.sync.dma_start(w2[:], w2_in.rearrange("g p e -> p g e"))
            awT = consts.tile([P, NG, P], F32)
            nc.sync.dma_start(awT[:], awT_in.rearrange("g p c -> p g c"))
            state_t = consts.tile([P, NG], F32)
            nc.sync.dma_start(state_t[:], state_in[:])
            ab_t = consts.tile([P, NG], F32)
            nc.sync.dma_start(ab_t[:], ab_in[:])
            if with_bias:
                ones_t = consts.tile([1, P], F32R)
                nc.sync.dma_start(ones_t[:], ones_in[:])
                bo_t = consts.tile([1, E], F32R)
                nc.sync.dma_start(bo_t[:], bo_in[:])

            # ---- a_state[(h,s), g] = A_w blockdiag^T @ state + A_b ----
            astate = consts.tile([P, NG], F32)
            for g in range(NG):
                psA = pmm1_pool.tile([P, STRIPE], F32, tag="pmm1")
                nc.tensor.matmul(
                    psA[:, 0:1], awT[:, g, :], state_t[:, g : g + 1],
                    start=True, stop=True,
                )
                nc.vector.tensor_add(
                    out=astate[:, g : g + 1], in0=psA[:, 0:1], in1=ab_t[:, g : g + 1]
                )

            for st in range(NSTRIPES):
                # ---- load x stripe ----
                x_t = xs.tile([P, TT_PER_STRIPE, E], F32, tag="x")
                nc.sync.dma_start(x_t[:], x_view[st])

                # ---- transpose x to feature-major, round to f32r ----
                xT = xts.tile([P, NG, STRIPE], F32R, tag="xT")
                for et in range(NG):
                    pstx = ptx_pool.tile([P, STRIPE], F32, tag="ptx")
                    for tt in range(TT_PER_STRIPE):
                        nc.tensor.transpose(
                            pstx[:, ts(tt, P)],
                            x_t[:, tt, ds(et * P, P)],
                            ident[:],
                        )
                    nc.vector.tensor_copy(xT[:, et, :], pstx[:])

                # ---- mm1 + a_state add -> nsT (fp32) and nsT_r (f32r) ----
                nsT = nsts.tile([P, NG, STRIPE], F32, tag="nsT")
                nsT_r = nsts.tile([P, NG, STRIPE], F32R, tag="nsT_r")
                for g in range(NG):
                    ps1 = pmm1_pool.tile([P, STRIPE], F32, tag="pmm1")
                    nc.tensor.matmul(
                        ps1[:], smT[:, g, :], xT[:, g, :], start=True, stop=True
                    )
                    nc.vector.tensor_scalar(
                        out=nsT[:, g, :], in0=ps1[:],
                        scalar1=astate[:, g : g + 1], scalar2=None,
                        op0=mybir.AluOpType.add,
                    )
                    nc.vector.tensor_scalar(
                        out=nsT_r[:, g, :], in0=ps1[:],
                        scalar1=astate[:, g : g + 1], scalar2=None,
                        op0=mybir.AluOpType.add,
                    )

                for tt in range(TT_PER_STRIPE):
                    t0 = st * STRIPE + tt * P

                    # ---- new_state back to token-major and out to HBM ----
                    ns_sb = stores.tile([P, H, S], F32, tag="ns")
                    for half in range(2):
                        pstn = ptns_pool.tile([P, STRIPE], F32, tag="ptns")
                        for j in range(4):
                            g = half * 4 + j
                            nc.tensor.transpose(
                                pstn[:, ts(j, P)],
                                nsT[:, g, ds(tt * P, P)],
                                ident[:],
                            )
                        nc.vector.tensor_copy(
                            ns_sb[:, ds(half * 8, 8), :].rearrange("t h s -> t (h s)"),
                            pstn[:],
                        )
                    nc.sync.dma_start(
                        ns_o[:, ds(t0, P), :].rearrange("h t s -> t h s"), ns_sb[:]
                    )

                    # ---- out = nsT_r.T @ W2 (+ bias) ----
                    out_sb = stores.tile([P, E], F32, tag="out")
                    for nch in range(2):
                        ps3 = pmm3_pool.tile([P, STRIPE], F32, tag="pmm3")
                        for g in range(NG):
                            nc.tensor.matmul(
                                ps3[:],
                                nsT_r[:, g, ds(tt * P, P)],
                                w2[:, g, ds(nch * STRIPE, STRIPE)],
                                start=(g == 0),
                                stop=(g == NG - 1 and not with_bias),
                            )
                        if with_bias:
                            nc.tensor.matmul(
                                ps3[:],
                                ones_t[:, :],
                                bo_t[:, ds(nch * STRIPE, STRIPE)],
                                start=False, stop=True,
                            )
                        nc.scalar.copy(out=out_sb[:, ds(nch * STRIPE, STRIPE)], in_=ps3[:])
                    nc.sync.dma_start(out_o[ds(t0, P), :], out_sb[:])

    nc.compile()
    return nc


def _prep_weights(A_w, A_b, C_w, C_b, sm, Wo, bo):
    """Host-side weight packing (all tiny; W2 fusion done in float64)."""
    A_w = np.asarray(A_w, np.float64)
    C_w = np.asarray(C_w, np.float64)
    sm = np.asarray(sm, np.float64)
    Wo = np.asarray(Wo, np.float64)
    bo = np.asarray(bo, np.float64)
    A_b = np.asarray(A_b, np.float64)
    C_b = np.asarray(C_b, np.float64)

    # block-diagonal sm^T: smT_bd[g, (hl*64+d), (hl*64+s)] = sm[2g+hl, s, d]
    smT_bd = np.zeros((NG, P, P), np.float32)
    awT_bd = np.zeros((NG, P, P), np.float32)
    for g in range(NG):
        for hl in range(2):
            h = 2 * g + hl
            sl = slice(hl * 64, hl * 64 + 64)
            smT_bd[g, sl, sl] = sm[h].T          # [d, s]
            awT_bd[g, sl, sl] = A_w[h].T         # [j, s]

    # fused W2[(h,s), e] = sum_d C_w[h,d,s] * Wo[e, h*64+d], packed per group
    Wo_r = Wo.reshape(E, H, D)                   # [e, h, d]
    W2 = np.einsum("hds,ehd->hse", C_w, Wo_r)    # [h, s, e]
    w2_pack = (
        W2.reshape(NG, 2, S, E).reshape(NG, P, E).astype(np.float32)
    )

    ab_cols = A_b.reshape(NG, P).T.astype(np.float32)        # [128, 8]
    bo_eff = (bo + Wo @ C_b.reshape(E)).astype(np.float32)   # [1024]
    return smT_bd, awT_bd, w2_pack, ab_cols, bo_eff


def kernel(x, state, A_w, A_b, C_w, C_b, sm, Wo, bo):
    x = np.ascontiguousarray(np.asarray(x, np.float32))
    state = np.asarray(state, np.float32)

    smT_bd, awT_bd, w2_pack, ab_cols, bo_eff = _prep_weights(
        A_w, A_b, C_w, C_b, sm, Wo, bo
    )
    with_bias = bool(np.any(bo_eff != 0.0))

    if with_bias not in _CACHE:
        _CACHE[with_bias] = _build_kernel(with_bias)
    nc = _CACHE[with_bias]

    ident = np.eye(P, dtype=np.float32)
    in_maps = []
    for c in range(NCORES):
        b = c // 2
        t0 = (c % 2) * T_LOC
        state_cols = (
            state[:, b, :].reshape(NG, P).T.astype(np.float32)
        )  # [(hl*64+j), g]
        m = {
            "x_in": np.ascontiguousarray(x[b, t0 : t0 + T_LOC, :]),
            "ident_in": ident,
            "smT_in": smT_bd,
            "w2_in": w2_pack,
            "awT_in": awT_bd,
            "state_in": state_cols,
            "ab_in": ab_cols,
        }
        if with_bias:
            m["ones_in"] = np.ones((1, P), np.float32)
            m["bo_in"] = bo_eff.reshape(1, E)
        in_maps.append(m)

    global _last_in_maps
    _last_in_maps = in_maps
    res = run_bass_kernel_spmd(nc, in_maps, core_ids=list(range(NCORES)))

    out_full = np.empty((B, T, E), np.float32)
    ns_full = np.empty((H, B, T, S), np.float32)
    for c in range(NCORES):
        b = c // 2
        t0 = (c % 2) * T_LOC
        out_full[b, t0 : t0 + T_LOC, :] = res.results[c]["out_o"]
        ns_full[:, b, t0 : t0 + T_LOC, :] = res.results[c]["ns_o"]

    return out_full, ns_full
